# revision 4
# baseline (speedup 1.0000x reference)
"""Trainium2 Bass kernel for nn_Attention_40149354283630.

Multi-head attention (16 heads, head_dim 64) with mixed 1D-latent + axial-2D
spatial RoPE, over x:(8, 1024, 1024). Data-parallel over the batch dim across
8 NeuronCores; each core runs the full transformer block for one batch element.

Per-core dataflow (all matmuls in float32r — full-rate, ~1.5e-4 rel error):
  xT [hid, s] (host-transposed)
  V  = (xT.T @ Wv + bv)            natural [s, dims]   (bias seeded into PSUM)
  QT = (Wq.T @ xT + bq)            transposed [dims, s], bias via tensor_scalar
  rope: QTrot = QT*COS + (PERM@QT)*SIN'   (pair-swap via permutation matmul)
  per head-pair t (2 heads per 128-partition tile):
    S^T[k,q] = KTrot.T @ QTrot     row-packed (contraction 64/head, groups 0/64)
    P^T      = exp(S^T / 8)        ScalarE, scale folded in, no max-subtraction
    [outT;sums] = [V|1].T @ P^T    M=65 augmented PV, sums ride in row 64
    nrm      = 1/(ones64 outer sums)  row-64-sourced broadcast matmul + recip
    attnT    = outT * nrm          partition-shifted write packs the head pair
  out = attnT.T @ Wo + bo          natural [s, hid]
"""

import numpy as np
from contextlib import ExitStack

import concourse.bass as bass
import concourse.tile as tile
from concourse import bacc, mybir
from concourse.bass_utils import run_bass_kernel_spmd

N_CORES = 8
HID, NH, HD = 1024, 16, 64
S = 1024
LAT, BASE = 16, 10000.0
NPAIR = 8  # head-pair tiles (2 heads x 64 dims = 128 partitions)

f32 = mybir.dt.float32
f32r = mybir.dt.float32r
EXP = mybir.ActivationFunctionType.Exp

_CACHE = {}


def _build_nc():
    nc = bacc.Bacc("TRN2", target_bir_lowering=False, debug=False, num_devices=N_CORES)

    def din(name, shape, dt):
        return nc.dram_tensor(name, shape, dt, kind="ExternalInput").ap()

    xT_d = din("xT", [HID, S], f32r)
    wq_d = din("wq", [HID, HID], f32r)
    wk_d = din("wk", [HID, HID], f32r)
    wv_d = din("wv", [HID, HID], f32r)
    wo_d = din("wo", [HID, HID], f32r)
    bqc_d = din("bqc", [128, 8], f32)
    bkc_d = din("bkc", [128, 8], f32)
    bvr_d = din("bvr", [1, HID], f32r)
    bor_d = din("bor", [1, HID], f32r)
    trig_d = din("trig", [128, 2 * S], f32)  # cols 0:S = COS, S:2S = SIN'
    perm_d = din("perm", [128, 128], f32r)
    ones_d = din("ones", [128, 128], f32r)
    augc_d = din("augc", [128, 16], f32r)
    out_d = nc.dram_tensor("out", [S, HID], f32, kind="ExternalOutput").ap()

    with tile.TileContext(nc) as tc, ExitStack() as ctx:
        # SBUF pools
        xt_p = ctx.enter_context(tc.tile_pool(name="xt", bufs=1))
        wsm_p = ctx.enter_context(tc.tile_pool(name="wsm", bufs=2))
        wb_p = ctx.enter_context(tc.tile_pool(name="wb", bufs=2))
        rot_p = ctx.enter_context(tc.tile_pool(name="rot", bufs=2))
        vst_p = ctx.enter_context(tc.tile_pool(name="vst", bufs=1))
        pt_p = ctx.enter_context(tc.tile_pool(name="pt", bufs=2))
        qtb_p = ctx.enter_context(tc.tile_pool(name="qtb", bufs=2))
        tt_p = ctx.enter_context(tc.tile_pool(name="tt", bufs=1))
        ssb_p = ctx.enter_context(tc.tile_pool(name="ssb", bufs=1))
        nrc_p = ctx.enter_context(tc.tile_pool(name="nrc", bufs=1))
        attn_p = ctx.enter_context(tc.tile_pool(name="attn", bufs=1))
        cst_p = ctx.enter_context(tc.tile_pool(name="cst", bufs=1))
        # PSUM pools
        flex_p = ctx.enter_context(tc.tile_pool(name="flex", bufs=1, space="PSUM"))
        prj_p = ctx.enter_context(tc.tile_pool(name="prj", bufs=2, space="PSUM"))
        pv_p = ctx.enter_context(tc.tile_pool(name="pv", bufs=1, space="PSUM"))

        # ---- constants ----
        trig = cst_p.tile([128, 2 * S], f32, tag="trig")
        nc.sync.dma_start(trig[:], trig_d[:])
        cos_t = trig[:, 0:S]
        sin_t = trig[:, S : 2 * S]
        perm = cst_p.tile([128, 128], f32r, tag="perm")
        nc.sync.dma_start(perm[:], perm_d[:])
        ones = cst_p.tile([128, 128], f32r, tag="ones")
        nc.sync.dma_start(ones[:], ones_d[:])
        bqc = cst_p.tile([128, 8], f32, tag="bqc")
        nc.sync.dma_start(bqc[:], bqc_d[:])
        bkc = cst_p.tile([128, 8], f32, tag="bkc")
        nc.sync.dma_start(bkc[:], bkc_d[:])
        bvr = cst_p.tile([1, HID], f32r, tag="bvr")
        nc.sync.dma_start(bvr[:], bvr_d[:])
        bor = cst_p.tile([1, HID], f32r, tag="bor")
        nc.sync.dma_start(bor[:], bor_d[:])

        # ---- xT resident ----
        xt = []
        for k in range(8):
            t = xt_p.tile([128, S], f32r, tag=f"xt{k}", name=f"xt{k}")
            nc.sync.dma_start(t[:], xT_d[k * 128 : (k + 1) * 128, :])
            xt.append(t)

        # ---- V projection (natural [s, dims+aug]), bias seeded in PSUM ----
        vst = []
        for st in range(8):
            v = vst_p.tile([128, 16 * 65], f32r, tag=f"vst{st}", name=f"vst{st}")
            # ones-augmentation columns (col 64 of each 65-block)
            nc.sync.dma_start(
                v[:].rearrange("p (h c) -> p h c", c=65)[:, :, 64:65],
                augc_d[:].unsqueeze(2),
            )
            vst.append(v)
        for c4 in range(4):
            wb = wb_p.tile([128, 8, 256], f32r, tag="wb", name=f"wbv{c4}")
            nc.sync.dma_start(
                wb[:],
                wv_d[:, c4 * 256 : (c4 + 1) * 256].rearrange("(a p) m -> p a m", p=128),
            )
            for st in range(8):
                ps = flex_p.tile([128, 256], f32, tag="flex", name=f"vps{c4}_{st}")
                nc.tensor.matmul(
                    ps[:],
                    ones[0:1, 0:128],
                    bvr[0:1, c4 * 256 : (c4 + 1) * 256],
                    start=True,
                    stop=False,
                )
                for k in range(8):
                    nc.tensor.matmul(
                        ps[:],
                        xt[k][:, st * 128 : (st + 1) * 128],
                        wb[:, k, :],
                        start=False,
                        stop=(k == 7),
                    )
                nc.vector.tensor_copy(
                    vst[st][:].rearrange("p (h c) -> p h c", c=65)[
                        :, 4 * c4 : 4 * c4 + 4, 0:64
                    ],
                    ps[:].rearrange("p (h c) -> p h c", c=64),
                )

        # ---- per head pair: Q/K proj + rope + attention ----
        attn = []
        for t in range(NPAIR):
            # -- Q and K projections + rope --
            rots = {}
            for which, w_d, bcol in (("q", wq_d, bqc), ("k", wk_d, bkc)):
                wsm = wsm_p.tile(
                    [128, 8, 128], f32r, tag=f"wsm{which}", name=f"w{which}{t}"
                )
                nc.sync.dma_start(
                    wsm[:],
                    w_d[:, t * 128 : (t + 1) * 128].rearrange("(a p) m -> p a m", p=128),
                )
                qtb = qtb_p.tile([128, S], f32r, tag="qtb", name=f"{which}tb{t}")
                for qc in range(2):
                    ps = prj_p.tile([128, 512], f32, tag="prj", name=f"{which}ps{t}_{qc}")
                    for k in range(8):
                        nc.tensor.matmul(
                            ps[:],
                            wsm[:, k, :],
                            xt[k][:, qc * 512 : (qc + 1) * 512],
                            start=(k == 0),
                            stop=(k == 7),
                        )
                    nc.vector.tensor_scalar(
                        qtb[:, qc * 512 : (qc + 1) * 512],
                        ps[:],
                        bcol[:, t : t + 1],
                        None,
                        op0=mybir.AluOpType.add,
                    )
                rot = rot_p.tile([128, S], f32r, tag=f"rot{which}", name=f"{which}rot{t}")
                for qc in range(2):
                    sl = slice(qc * 512, (qc + 1) * 512)
                    sw = prj_p.tile([128, 512], f32, tag="prj", name=f"{which}sw{t}_{qc}")
                    nc.tensor.matmul(sw[:], perm[:], qtb[:, sl], start=True, stop=True)
                    ta = tt_p.tile([128, 512], f32, tag="ta", name=f"{which}ta{t}_{qc}")
                    nc.vector.tensor_mul(ta[:], qtb[:, sl].bitcast(f32), cos_t[:, sl])
                    tb = tt_p.tile([128, 512], f32, tag="tb", name=f"{which}tb2{t}_{qc}")
                    nc.vector.tensor_mul(tb[:], sw[:], sin_t[:, sl])
                    nc.vector.tensor_add(rot[:, sl], ta[:], tb[:])
                rots[which] = rot
            qrot, krot = rots["q"], rots["k"]

            # -- scores + exp + augmented PV, streaming over k-tiles --
            pvt = [
                pv_p.tile([65, S], f32, tag="pvh0", name=f"pv{t}_0"),
                pv_p.tile([65, S], f32, tag="pvh1", name=f"pv{t}_1"),
            ]
            for kt in range(8):
                ksl = slice(kt * 128, (kt + 1) * 128)
                ph_pair = []
                for h, (pr, tp) in enumerate(
                    ((slice(0, 64), (0, 0)), (slice(64, 128), (64, 0)))
                ):
                    sc = flex_p.tile([128, S], f32, tag="flex", name=f"sc{t}_{kt}_{h}")
                    for qc in range(2):
                        qsl = slice(qc * 512, (qc + 1) * 512)
                        nc.tensor.matmul(
                            sc[:, qsl],
                            krot[pr, ksl],
                            qrot[pr, qsl],
                            start=True,
                            stop=True,
                            tile_position=tp,
                        )
                    ph = pt_p.tile([128, S], f32r, tag=f"ph{h}", name=f"ph{t}_{kt}_{h}")
                    nc.scalar.activation(ph[:], sc[:], EXP, scale=0.125)
                    ph_pair.append(ph)
                for h, ph in enumerate(ph_pair):
                    vsl = slice((2 * t + h) * 65, (2 * t + h) * 65 + 65)
                    for qc in range(2):
                        qsl = slice(qc * 512, (qc + 1) * 512)
                        nc.tensor.matmul(
                            pvt[h][:, qsl],
                            vst[kt][:, vsl],
                            ph[:, qsl],
                            start=(kt == 0),
                            stop=(kt == 7),
                        )

            # -- normalization (no DRAM bounce):
            #    sums sit in pv row 64 -> copy to SBUF row 0 (partition shift)
            #    -> broadcast to 64 rows via ones outer-product matmul
            #    -> reciprocal (PSUM->SBUF evict) -> multiply
            ssb = ssb_p.tile([1, 2 * S], f32r, tag="ssb", name=f"ssb{t}")
            at = attn_p.tile([128, S], f32r, tag=f"attn{t}", name=f"attn{t}")
            for h in range(2):
                nc.scalar.copy(ssb[0:1, h * S : h * S + S], pvt[h][64:65, :])
            for h in range(2):
                pn = flex_p.tile([64, S], f32, tag="flex", name=f"pn{t}_{h}")
                for qc in range(2):
                    qsl = slice(qc * 512, (qc + 1) * 512)
                    nc.tensor.matmul(
                        pn[:, qsl],
                        ones[0:1, 0:64],
                        ssb[0:1, h * S + qc * 512 : h * S + (qc + 1) * 512],
                        start=True,
                        stop=True,
                    )
                nr = nrc_p.tile([64, S], f32, tag=f"nrc{h}", name=f"nr{t}_{h}")
                nc.vector.reciprocal(nr[:], pn[:])
                # attnT rows h*64..h*64+64  <-  out_un rows 0:64 of pvt[h]
                nc.vector.tensor_mul(
                    at[h * 64 : h * 64 + 64, :], pvt[h][0:64, :], nr[:]
                )
            attn.append(at)

        # ---- output projection ----
        for c4 in range(4):
            wb = wb_p.tile([128, 8, 256], f32r, tag="wb", name=f"wbo{c4}")
            nc.sync.dma_start(
                wb[:],
                wo_d[:, c4 * 256 : (c4 + 1) * 256].rearrange("(a p) m -> p a m", p=128),
            )
            for qt in range(8):
                ps = flex_p.tile([128, 256], f32, tag="flex", name=f"ops{c4}_{qt}")
                nc.tensor.matmul(
                    ps[:],
                    ones[0:1, 0:128],
                    bor[0:1, c4 * 256 : (c4 + 1) * 256],
                    start=True,
                    stop=False,
                )
                for dt in range(8):
                    nc.tensor.matmul(
                        ps[:],
                        attn[dt][:, qt * 128 : (qt + 1) * 128],
                        wb[:, dt, :],
                        start=False,
                        stop=(dt == 7),
                    )
                ostg = tt_p.tile([128, 256], f32, tag="ta", name=f"ostg{c4}_{qt}")
                nc.vector.tensor_copy(ostg[:], ps[:])
                nc.sync.dma_start(
                    out_d[qt * 128 : (qt + 1) * 128, c4 * 256 : (c4 + 1) * 256],
                    ostg[:],
                )

    nc.compile()
    return nc


def _trig_tables(height, width):
    """COS / SIN' tables in [dim, s] pair-tile layout, fp32 arithmetic to match
    the reference's fp32 angle computation."""
    s = np.arange(S, dtype=np.float32)
    ang = np.zeros((64, S), dtype=np.float32)
    inv1 = (
        1.0 / (BASE ** (np.arange(0, LAT, 2, dtype=np.float32) / np.float32(LAT)))
    ).astype(np.float32)
    half = 24
    inv2 = (
        1.0 / (BASE ** (np.arange(0, half, 2, dtype=np.float32) / np.float32(half)))
    ).astype(np.float32)
    col = (np.arange(S) % width).astype(np.float32)
    row = (np.arange(S) // width).astype(np.float32)
    for j in range(8):
        a = (s * inv1[j]).astype(np.float32)
        ang[2 * j] = a
        ang[2 * j + 1] = a
    for j in range(12):
        a = (col * inv2[j]).astype(np.float32)
        ang[16 + 2 * j] = a
        ang[16 + 2 * j + 1] = a
        b = (row * inv2[j]).astype(np.float32)
        ang[40 + 2 * j] = b
        ang[40 + 2 * j + 1] = b
    cos64 = np.cos(ang).astype(np.float32)
    sin64 = np.sin(ang).astype(np.float32)
    sgn = np.tile(np.array([[-1.0], [1.0]], np.float32), (32, 1))
    sinp = (sin64 * sgn).astype(np.float32)
    cos128 = np.concatenate([cos64, cos64], axis=0)
    sinp128 = np.concatenate([sinp, sinp], axis=0)
    return np.concatenate([cos128, sinp128], axis=1).astype(np.float32)


def _static_consts(bq, bk):
    permm = np.zeros((128, 128), np.float32)
    for k in range(128):
        partner = k + 1 if k % 2 == 0 else k - 1
        permm[k, partner] = 1.0
    return {
        "perm": permm,
        "ones": np.ones((128, 128), np.float32),
        "augc": np.ones((128, 16), np.float32),
        "bqc": np.ascontiguousarray(bq.reshape(8, 128).T).astype(np.float32),
        "bkc": np.ascontiguousarray(bk.reshape(8, 128).T).astype(np.float32),
    }


def _run(inputs, trace):
    x = np.asarray(inputs["x"], np.float32)
    Wq = np.ascontiguousarray(np.asarray(inputs["Wq"], np.float32))
    Wk = np.ascontiguousarray(np.asarray(inputs["Wk"], np.float32))
    Wv = np.ascontiguousarray(np.asarray(inputs["Wv"], np.float32))
    Wo = np.ascontiguousarray(np.asarray(inputs["Wo"], np.float32))
    bq = np.asarray(inputs["bq"], np.float32)
    bk = np.asarray(inputs["bk"], np.float32)
    bv = np.asarray(inputs["bv"], np.float32)
    bo = np.asarray(inputs["bo"], np.float32)
    height = int(inputs["height"])
    width = int(inputs["width"])
    B = x.shape[0]
    assert B == N_CORES and x.shape[1] == S and x.shape[2] == HID

    if "nc" not in _CACHE:
        _CACHE["nc"] = _build_nc()
    nc = _CACHE["nc"]

    trig = _trig_tables(height, width)
    consts = _static_consts(bq, bk)
    in_maps = []
    for c in range(N_CORES):
        m = {
            "xT": np.ascontiguousarray(x[c].T),
            "wq": Wq,
            "wk": Wk,
            "wv": Wv,
            "wo": Wo,
            "bqc": consts["bqc"],
            "bkc": consts["bkc"],
            "bvr": np.ascontiguousarray(bv.reshape(1, HID)),
            "bor": np.ascontiguousarray(bo.reshape(1, HID)),
            "trig": trig,
            "perm": consts["perm"],
            "ones": consts["ones"],
            "augc": consts["augc"],
        }
        in_maps.append(m)
    res = run_bass_kernel_spmd(nc, in_maps, list(range(N_CORES)), trace=trace)
    y = np.stack([res.results[c]["out"] for c in range(N_CORES)], axis=0)
    return y.astype(np.float32), res.exec_time_ns


def kernel(**inputs):
    y, _ = _run(inputs, trace=False)
    return y


def kernel_profiled(**inputs):
    """Like kernel() but also returns NTFF-profiled HW exec time (ns)."""
    try:
        import ntff_hook

        ntff_hook.install()
    except ImportError:
        pass
    return _run(inputs, trace=True)
